# revision 1
# baseline (speedup 1.0000x reference)
"""GAT block (GATConv + InstanceNorm + residual + ELU) on 8 Trainium2 NeuronCores.

Strategy (graph/data parallel over dst nodes):
  - Host routes each edge to the core owning its dst node; per core, dst
    nodes are sorted by (degree, src<HALF-degree) and grouped into tiles of
    128 (dst node == partition, so aggregation needs no scatter).
  - Incoming edges of a tile live in padded slot columns: k=0 is the self
    loop (filled from on-chip hx_own, no gather), then group-A slots
    (src < HALF) and group-B slots (src >= HALF).
  - Slot rows are fetched with ONE batched dma_gather ucode instruction per
    (tile, half) from a [N+2, 192]-f32 table hx192 = x @ [W|w_src|w_dst|0]
    that each core builds locally (int16 gather indices fit because each
    half has < 32768 rows; row 0 / row HALF+1 are -1e30 dummy rows that
    softmax kills, used for padding slots).
  - Softmax over slots skips the segment max (logits are bounded, exp is
    clamped at -88 so it cannot overflow; result is mathematically equal).
  - a_edge = edge_attr @ v (v folded on host) via TensorE on a
    host-transposed 4-slot-interleaved eaT4 layout; the self loop's a_edge
    is (sum_k a_edge_k) / deg (linearity in edge_attr).
  - InstanceNorm stats via ones-matmul partition reduction, AllReduce'd
    across the 8 cores; finalize = per-channel affine + residual + ELU.
"""

import math
import numpy as np

P = 128


def _cfg_full():
    return dict(N=50000, E=1600000, F=128, H=8, Dh=16, ED=16, NC=8)


def _half(N):
    # multiple of 128 so Phase-A chunks never span the A/B table boundary;
    # both halves must stay < 32768 rows (int16 gather indices).
    h = ((N // 2) // P) * P
    assert h <= 32767 and (N - h) <= 32766
    return h


def _fold_weights(W, att_src, att_dst, W_e, att_edge, H, Dh, FX):
    F = W.shape[0]
    w_src = np.stack(
        [W[:, h * Dh:(h + 1) * Dh] @ att_src[h] for h in range(H)], axis=1)
    w_dst = np.stack(
        [W[:, h * Dh:(h + 1) * Dh] @ att_dst[h] for h in range(H)], axis=1)
    Wb = np.zeros((F, FX), dtype=np.float32)
    Wb[:, :F] = W
    Wb[:, F:F + H] = w_src
    Wb[:, F + H:F + 2 * H] = w_dst
    v = np.stack(
        [W_e[:, h * Dh:(h + 1) * Dh] @ att_edge[h] for h in range(H)], axis=1
    ).astype(np.float32)
    ED = W_e.shape[0]
    v4 = np.zeros((4 * ED, 4 * H), dtype=np.float32)
    for j in range(4):
        v4[j * ED:(j + 1) * ED, j * H:(j + 1) * H] = v
    return Wb, v4


def _preprocess(edge_index, edge_attr, cfg):
    N, ED, NC = cfg["N"], cfg["ED"], cfg["NC"]
    HALF = _half(N)
    Np = N // NC
    n_tiles = math.ceil(Np / P)
    src = np.asarray(edge_index[0]).astype(np.int64)
    dst = np.asarray(edge_index[1]).astype(np.int64)
    ea = np.asarray(edge_attr, dtype=np.float32)

    cores = []
    for c in range(NC):
        m = (dst >= c * Np) & (dst < (c + 1) * Np)
        e_ids = np.nonzero(m)[0]
        dst_c = dst[e_ids] - c * Np
        is_a = src[e_ids] < HALF
        # sort edges by (dst, group) so each node's A-edges precede B-edges
        order_e = np.lexsort((~is_a, dst_c))
        e_ids = e_ids[order_e]
        dst_c = dst_c[order_e]
        deg = np.bincount(dst_c, minlength=Np).astype(np.int64)
        degA = np.bincount(dst_c[src[e_ids] < HALF], minlength=Np).astype(np.int64)
        cum = np.zeros(Np + 1, dtype=np.int64)
        np.cumsum(deg, out=cum[1:])
        node_order = np.lexsort((-degA, -deg))
        pad_nodes = n_tiles * P - Np
        node_order_p = np.concatenate(
            [node_order, np.full(pad_nodes, -1, dtype=np.int64)])
        KAs, KBs = [], []
        for t in range(n_tiles):
            nt = node_order_p[t * P:(t + 1) * P]
            real = nt[nt >= 0]
            if len(real):
                KAs.append(int(degA[real].max()))
                KBs.append(int((deg[real] - degA[real]).max()))
            else:
                KAs.append(0)
                KBs.append(0)
        cores.append(dict(e_ids=e_ids, dst_c=dst_c, deg=deg, degA=degA,
                          cum=cum, node_order=node_order_p, KAs=KAs, KBs=KBs))

    # CA = self col + group-A slots (padded to 4); CB = group-B slots
    CAs, CBs = [], []
    for t in range(n_tiles):
        ka = max(c["KAs"][t] for c in cores)
        kb = max(c["KBs"][t] for c in cores)
        CAs.append(((1 + ka + 3) // 4) * 4)
        CBs.append(max(((kb + 3) // 4) * 4, 4))
    CAarr = np.array(CAs, dtype=np.int64)
    offsA = np.zeros(n_tiles + 1, dtype=np.int64)
    np.cumsum((CAarr - 1) * P, out=offsA[1:])       # gathered A slots
    offsB = np.zeros(n_tiles + 1, dtype=np.int64)
    np.cumsum(np.array(CBs, dtype=np.int64) * P, out=offsB[1:])
    offs4 = np.zeros(n_tiles + 1, dtype=np.int64)   # eaT4 quad-column offsets
    np.cumsum((CAarr + np.array(CBs)) // 4 * P, out=offs4[1:])
    SA, SB = int(offsA[-1]), int(offsB[-1])

    for c in range(NC):
        st = cores[c]
        deg, degA, cum = st["deg"], st["degA"], st["cum"]
        node_order = st["node_order"]
        idxA = np.zeros(SA, dtype=np.int16)   # 0 -> dummy-A row (j-order)
        idxB = np.zeros(SB, dtype=np.int16)   # 0 -> dummy-B row
        eaT4 = np.zeros((4 * ED, int(offs4[-1])), dtype=np.float32)
        rdeg = np.ones(n_tiles * P, dtype=np.float32)
        tile_of_pos = np.repeat(np.arange(n_tiles), P)
        p_of_pos = np.tile(np.arange(P), n_tiles)
        real_m = node_order >= 0
        nodes = node_order[real_m]
        rdeg[real_m] = 1.0 / np.maximum(deg[nodes], 1).astype(np.float32)
        pos_r = np.nonzero(real_m)[0]
        pos_of_node = np.empty(Np, dtype=np.int64)
        pos_of_node[nodes] = pos_r
        nloc = st["dst_c"]
        e_pos = pos_of_node[nloc]
        e_t = tile_of_pos[e_pos]
        e_p = p_of_pos[e_pos]
        r_in_node = np.arange(len(nloc)) - cum[nloc]   # 0..deg-1, A first
        e_srcs = src[st["e_ids"]]
        in_a = e_srcs < HALF
        rA = r_in_node
        rB = r_in_node - degA[nloc]
        jA = offsA[e_t[in_a]] + rA[in_a] * P + e_p[in_a]
        idxA[jA] = (e_srcs[in_a] + 1).astype(np.int16)
        jB = offsB[e_t[~in_a]] + rB[~in_a] * P + e_p[~in_a]
        idxB[jB] = (e_srcs[~in_a] - HALF + 1).astype(np.int16)

        # dma_gather SBUF index layout: value j at [j%16, j//16], the 16-row
        # block replicated 8x down the partitions (one copy per Q7 core pair)
        def _pack16(flat):
            cols = len(flat) // 16
            out2 = np.zeros((P, max(cols, 1)), dtype=np.int16)
            if cols:
                out2[:] = np.tile(flat.reshape(-1, 16).T, (8, 1))
            return out2
        # eaT4: group A edge -> in-group col 1+rA; group B edge -> col rB;
        # B quad block follows A quad block within each tile
        kg = np.where(in_a, 1 + rA, rB)
        qoff = np.where(in_a, 0, CAarr[e_t] // 4)
        col = offs4[e_t] + (qoff + (kg >> 2)) * P + e_p
        jj = (kg & 3).astype(np.int64)
        ea_c = ea[st["e_ids"]]
        for j4 in range(4):
            mj = jj == j4
            eaT4[j4 * ED:(j4 + 1) * ED, col[mj]] = ea_c[mj].T
        st["in"] = dict(idxA=_pack16(idxA), idxB=_pack16(idxB),
                        eaT4=eaT4, rdeg=rdeg)
    return cores, dict(CAs=CAs, CBs=CBs, offs4=offs4,
                       offsA=offsA, offsB=offsB, HALF=HALF)


# ---------------------------------------------------------------- device
def _build(cfg, meta, finalize=True):
    import concourse.bass as bass
    import concourse.bacc as bacc
    import concourse.tile as tile
    from concourse import mybir

    N, F, H, ED, NC = cfg["N"], cfg["F"], cfg["H"], cfg["ED"], cfg["NC"]
    Np = N // NC
    CAs, CBs = meta["CAs"], meta["CBs"]
    offs4 = meta["offs4"]
    offsA, offsB = meta["offsA"], meta["offsB"]
    HALF = meta["HALF"]
    n_tiles = len(CAs)
    FX = 192                 # table row width (f32): 768B, %256 for dma_gather
    FU = F + 2 * H           # used columns
    SA, SB = int(offsA[-1]), int(offsB[-1])
    f32 = mybir.dt.float32
    i16 = mybir.dt.int16
    AF = mybir.ActivationFunctionType
    OP = mybir.AluOpType
    EPS_IN, NEG = 1e-5, 0.2

    nc = bacc.Bacc("TRN2", target_bir_lowering=False, debug=False,
                   num_devices=NC)
    xT_d = nc.declare_dram_parameter("xT", [F, N], f32, isOutput=False)
    xTo_d = nc.declare_dram_parameter("xTo", [F, n_tiles * P], f32,
                                      isOutput=False)
    xo_d = nc.declare_dram_parameter("xo", [n_tiles * P, F], f32,
                                     isOutput=False)
    Wb_d = nc.declare_dram_parameter("Wb", [F, FX], f32, isOutput=False)
    v4_d = nc.declare_dram_parameter("v4", [4 * ED, 4 * H], f32, isOutput=False)
    ixA_d = nc.declare_dram_parameter("idxA", [P, max(SA // 16, 1)], i16,
                                      isOutput=False)
    ixB_d = nc.declare_dram_parameter("idxB", [P, max(SB // 16, 1)], i16,
                                      isOutput=False)
    ea4_d = nc.declare_dram_parameter("eaT4", [4 * ED, int(offs4[-1])], f32,
                                      isOutput=False)
    rdeg_d = nc.declare_dram_parameter("rdeg", [n_tiles * P], f32,
                                       isOutput=False)
    gam_d = nc.declare_dram_parameter("gamma", [F], f32, isOutput=False)
    bet_d = nc.declare_dram_parameter("beta", [F], f32, isOutput=False)
    out_d = nc.declare_dram_parameter("out", [n_tiles * P, F], f32,
                                      isOutput=True)

    with tile.TileContext(nc) as tc:
        with (
            tc.tile_pool(name="dram", bufs=1, space="DRAM") as dram,
            tc.tile_pool(name="consts", bufs=1) as consts,
            tc.tile_pool(name="ph_a", bufs=3) as pha,
            tc.tile_pool(name="ph_a_ps", bufs=2, space="PSUM") as pha_ps,
            tc.tile_pool(name="ph_b", bufs=2) as phb,
            tc.tile_pool(name="ph_b_ps", bufs=2, space="PSUM") as phb_ps,
            tc.tile_pool(name="stats_ps", bufs=2, space="PSUM") as stats_ps,
            tc.tile_pool(name="keep", bufs=1) as keep,
        ):
            hx = dram.tile([N + 2, FX], f32)

            Wb_s = consts.tile([F, FX], f32)
            nc.sync.dma_start(out=Wb_s[:], in_=Wb_d[:, :])
            v4_s = consts.tile([4 * ED, 4 * H], f32)
            nc.sync.dma_start(out=v4_s[:], in_=v4_d[:, :])
            ones = consts.tile([P, 1], f32)
            nc.vector.memset(ones[:], 1.0)

            # ---------------- Phase A: hx = x @ Wb  (full table, per core)
            # table rows: 0 dummy-A | 1..HALF nodes 0..HALF-1 |
            #             HALF+1 dummy-B | HALF+2.. nodes HALF..N-1
            n_chunks = math.ceil(N / P)
            for i in range(n_chunks):
                r0 = i * P
                nrow = min(P, N - r0)
                trow = r0 + 1 if r0 < HALF else r0 + 2
                xT_t = pha.tile([F, P], f32, name="xT_t")
                nc.sync.dma_start(out=xT_t[:, :nrow], in_=xT_d[:, r0:r0 + nrow])
                hx_p = pha_ps.tile([P, FX], f32, name="hx_p")
                nc.tensor.matmul(out=hx_p[:], lhsT=xT_t[:], rhs=Wb_s[:],
                                 start=True, stop=True)
                hx_s = pha.tile([P, FX], f32, name="hx_s")
                nc.vector.tensor_copy(out=hx_s[:], in_=hx_p[:])
                nc.sync.dma_start(out=hx[trow:trow + nrow, :], in_=hx_s[:nrow, :])
            dum = pha.tile([1, FX], f32, name="dum")
            nc.vector.memset(dum[:], 0.0)
            nc.vector.memset(dum[:, F:F + H], -1e30)
            nc.sync.dma_start(out=hx[0:1, :], in_=dum[:])
            nc.sync.dma_start(out=hx[HALF + 1:HALF + 2, :], in_=dum[:])

            # hx_own: own nodes in tile order (for self-loop slot + a_dst)
            hx_own = keep.tile([P, n_tiles, FU], f32)
            for t in range(n_tiles):
                xTo_t = pha.tile([F, P], f32, name="xTo_t")
                nc.sync.dma_start(out=xTo_t[:], in_=xTo_d[:, t * P:(t + 1) * P])
                ho_p = pha_ps.tile([P, FX], f32, name="ho_p", tag="hx_p")
                nc.tensor.matmul(out=ho_p[:], lhsT=xTo_t[:], rhs=Wb_s[:],
                                 start=True, stop=True)
                nc.vector.tensor_copy(out=hx_own[:, t, :], in_=ho_p[:, :FU])

            # ---------------- Phase B: per-tile attention + aggregation
            out_all = keep.tile([P, n_tiles, F], f32)
            acc = keep.tile([P, 2], f32)
            nc.vector.memset(acc[:], 0.0)

            GMAX = 8  # dma_gather caps out at ~1024 indices/instruction
            for t in range(n_tiles):
                CA, CB = CAs[t], CBs[t]
                den_acc = phb.tile([P, H], f32, name="den_acc", tag="den_acc")
                msg_acc = phb.tile([P, F], f32, name="msg_acc", tag="msg_acc")
                aeL_B = phb.tile([P, H], f32, name="aeL_B", tag="aeL_B")
                rdeg_t = phb.tile([P, 1], f32, name="rdeg_t", tag="rdeg_t")
                nc.sync.dma_start(out=rdeg_t[:],
                                  in_=rdeg_d[t * P:(t + 1) * P, None])

                # two passes: group B first (accumulators init), then group A
                # (self-loop col 0, needs aeL_B for the self a_edge)
                for is_a in (False, True):
                    C = CA if is_a else CB
                    C4 = C // 4
                    g = phb.tile([P, C, FX], f32, name="g", tag="g")
                    if is_a:
                        nc.vector.tensor_copy(out=g[:, 0, :FU],
                                              in_=hx_own[:, t, :])
                        ng = (C - 1) * P
                        o0, o1 = int(offsA[t]) // 16, int(offsA[t + 1]) // 16
                        ix_t = phb.tile([P, max(ng // 16, 1)], i16,
                                        name="ix_t", tag="ix_t")
                        if ng:
                            nc.sync.dma_start(out=ix_t[:, :],
                                              in_=ixA_d[:, o0:o1])
                        src_ap = hx[:, :]
                        gc0 = 1
                        q0 = 0
                    else:
                        ng = C * P
                        o0, o1 = int(offsB[t]) // 16, int(offsB[t + 1]) // 16
                        ix_t = phb.tile([P, max(ng // 16, 1)], i16,
                                        name="ix_t", tag="ix_t")
                        if ng:
                            nc.sync.dma_start(out=ix_t[:, :],
                                              in_=ixB_d[:, o0:o1])
                        src_ap = hx[HALF + 1:, :]
                        gc0 = 0
                        q0 = CA // 4
                    ncols = ng // P
                    for g0 in range(0, ncols, GMAX):
                        kk = min(GMAX, ncols - g0)
                        nc.gpsimd.dma_gather(
                            out_ap=g[:, gc0 + g0:gc0 + g0 + kk, :],
                            in_ap=src_ap,
                            idxs_ap=ix_t[:, g0 * 8:(g0 + kk) * 8],
                            num_idxs=kk * P,
                            num_idxs_reg=kk * P,
                            elem_size=FX,
                        )
                    ea4_t = phb.tile([4 * ED, C4 * P], f32, name="ea4_t",
                                     tag="ea4_t")
                    nc.sync.dma_start(
                        out=ea4_t[:],
                        in_=ea4_d[:, int(offs4[t]) + q0 * P:
                                  int(offs4[t]) + (q0 + C4) * P])

                    # a_edge: quad matmuls [4ED,P] @ [4ED,4H]
                    ae = phb.tile([P, C, H], f32, name="ae", tag="ae")
                    QG = 16
                    for qg in range(math.ceil(C4 / QG)):
                        nq = min(QG, C4 - qg * QG)
                        ae_p = phb_ps.tile([P, QG * 4 * H], f32, name="ae_p",
                                           tag="ae_p")
                        for qi in range(nq):
                            q = qg * QG + qi
                            nc.tensor.matmul(
                                out=ae_p[:, qi * 4 * H:(qi + 1) * 4 * H],
                                lhsT=ea4_t[:, q * P:(q + 1) * P],
                                rhs=v4_s[:],
                                start=True, stop=True)
                        nc.vector.tensor_copy(
                            out=ae[:, qg * QG * 4:qg * QG * 4 + nq * 4, :],
                            in_=ae_p[:, :nq * 4 * H])
                    aeL = phb.tile([P, H], f32, name="aeL", tag="aeL")
                    nc.vector.tensor_reduce(
                        out=aeL[:], in_=ae.transpose([0, 2, 1]),
                        axis=mybir.AxisListType.X, op=OP.add)
                    if not is_a:
                        nc.vector.tensor_copy(out=aeL_B[:], in_=aeL[:])
                    else:
                        # self-loop a_edge = (sum of a_edge over ALL slots)/deg
                        nc.vector.tensor_add(aeL[:], aeL[:], aeL_B[:])
                        nc.vector.tensor_scalar_mul(ae[:, 0, :], aeL[:],
                                                    rdeg_t[:])

                    # logits -> exp(leaky) ; no segment max (clamped at -88)
                    al = phb.tile([P, H, C], f32, name="al", tag="al")
                    alv = al.transpose([0, 2, 1])
                    nc.vector.tensor_tensor(
                        out=alv, in0=g[:, :, F:F + H], in1=ae[:, :, :],
                        op=OP.add)
                    adst = hx_own[:, t, F + H:F + 2 * H]
                    nc.vector.tensor_tensor(
                        out=alv, in0=alv,
                        in1=adst.unsqueeze(1).broadcast_to((P, C, H)),
                        op=OP.add)
                    tl = phb.tile([P, H, C], f32, name="tl", tag="tl")
                    nc.vector.tensor_scalar_mul(tl[:], al[:], NEG)
                    nc.vector.tensor_tensor(out=al[:], in0=al[:], in1=tl[:],
                                            op=OP.max)
                    nc.vector.tensor_scalar_max(al[:], al[:], -88.0)
                    nc.scalar.activation(out=al[:], in_=al[:], func=AF.Exp)
                    # accumulate denominator and weighted messages
                    if not is_a:
                        nc.vector.tensor_reduce(
                            out=den_acc[:], in_=al[:],
                            axis=mybir.AxisListType.X, op=OP.add)
                    else:
                        den_t = phb.tile([P, H], f32, name="den_t",
                                         tag="den_t")
                        nc.vector.tensor_reduce(
                            out=den_t[:], in_=al[:],
                            axis=mybir.AxisListType.X, op=OP.add)
                        nc.vector.tensor_add(den_acc[:], den_acc[:], den_t[:])
                    gh = g[:, :, 0:F].rearrange("p k (h d) -> p k h d", h=H)
                    nc.vector.tensor_tensor(
                        out=gh, in0=gh,
                        in1=al.transpose([0, 2, 1]).unsqueeze(3)
                            .broadcast_to((P, C, H, F // H)),
                        op=OP.mult)
                    if not is_a:
                        nc.vector.tensor_reduce(
                            out=msg_acc[:],
                            in_=g[:, :, 0:F].transpose([0, 2, 1]),
                            axis=mybir.AxisListType.X, op=OP.add)
                    else:
                        msg_t = phb.tile([P, F], f32, name="msg_t",
                                         tag="msg_t")
                        nc.vector.tensor_reduce(
                            out=msg_t[:],
                            in_=g[:, :, 0:F].transpose([0, 2, 1]),
                            axis=mybir.AxisListType.X, op=OP.add)
                        nc.vector.tensor_add(msg_acc[:], msg_acc[:], msg_t[:])

                # out_pre = msg / den  (per-node alpha normalization)
                rec = phb.tile([P, H], f32, name="rec", tag="rec")
                nc.vector.tensor_scalar_add(rec[:], den_acc[:], 1e-16)
                nc.vector.reciprocal(rec[:], rec[:])
                op_t = out_all[:, t, :]
                nc.vector.tensor_tensor(
                    out=op_t.rearrange("p (h d) -> p h d", h=H),
                    in0=msg_acc.rearrange("p (h d) -> p h d", h=H),
                    in1=rec.unsqueeze(2).broadcast_to((P, H, F // H)),
                    op=OP.mult)

                # stats: per-channel sum & sumsq via ones-matmul
                sq = phb.tile([P, F], f32, name="sq", tag="sq")
                nc.vector.tensor_mul(sq[:], op_t, op_t)
                st_p = stats_ps.tile([P, 2], f32, name="st_p", tag="st_p")
                nc.tensor.matmul(out=st_p[:, 0:1], lhsT=op_t, rhs=ones[:],
                                 start=True, stop=True)
                nc.tensor.matmul(out=st_p[:, 1:2], lhsT=sq[:], rhs=ones[:],
                                 start=True, stop=True)
                nc.vector.tensor_add(acc[:], acc[:], st_p[:])

            # ---------------- Phase C: stats allreduce + normalize + ELU
            st_in = dram.tile([P, 2], f32)
            st_out = dram.tile([P, 2], f32, addr_space="Shared")
            nc.sync.dma_start(out=st_in[:], in_=acc[:])
            nc.gpsimd.collective_compute(
                "AllReduce", OP.add,
                replica_groups=[list(range(NC))],
                ins=[st_in[:].opt()], outs=[st_out[:].opt()])
            sg = keep.tile([P, 2], f32)
            nc.sync.dma_start(out=sg[:], in_=st_out[:])
            mean = keep.tile([P, 1], f32)
            nc.vector.tensor_scalar_mul(mean[:], sg[:, 0:1], 1.0 / N)
            ex2 = keep.tile([P, 1], f32)
            nc.vector.tensor_scalar_mul(ex2[:], sg[:, 1:2], 1.0 / N)
            var = keep.tile([P, 1], f32)
            nc.vector.tensor_mul(var[:], mean[:], mean[:])
            nc.vector.tensor_sub(var[:], ex2[:], var[:])
            rstd = keep.tile([P, 1], f32)
            eps_t = keep.tile([P, 1], f32)
            nc.vector.memset(eps_t[:], EPS_IN)
            nc.scalar.activation(out=rstd[:], in_=var[:], func=AF.Sqrt,
                                 bias=eps_t[:])
            nc.vector.reciprocal(rstd[:], rstd[:])
            gam_s = keep.tile([P, 1], f32)
            nc.sync.dma_start(out=gam_s[:], in_=gam_d[:, None])
            bet_s = keep.tile([P, 1], f32)
            nc.sync.dma_start(out=bet_s[:], in_=bet_d[:, None])
            scl = keep.tile([P, 1], f32)
            nc.vector.tensor_mul(scl[:], rstd[:], gam_s[:])
            bia = keep.tile([P, 1], f32)
            nc.vector.tensor_mul(bia[:], mean[:], scl[:])
            nc.vector.tensor_sub(bia[:], bet_s[:], bia[:])
            sb_dram = dram.tile([2, P], f32)
            nc.sync.dma_start(out=sb_dram[0, :], in_=scl[:, 0])
            nc.sync.dma_start(out=sb_dram[1, :], in_=bia[:, 0])
            sclB = keep.tile([P, F], f32)
            nc.sync.dma_start(out=sclB[:],
                              in_=sb_dram[0:1, :].broadcast_to((P, P)))
            biaB = keep.tile([P, F], f32)
            nc.sync.dma_start(out=biaB[:],
                              in_=sb_dram[1:2, :].broadcast_to((P, P)))

            with tc.tile_pool(name="ph_c", bufs=3) as phc:
                for t in range(n_tiles):
                    xo_t = phc.tile([P, F], f32, name="xo_t")
                    nc.sync.dma_start(out=xo_t[:],
                                      in_=xo_d[t * P:(t + 1) * P, :])
                    z = phc.tile([P, F], f32, name="z")
                    nc.vector.tensor_mul(z[:], out_all[:, t, :], sclB[:])
                    nc.vector.tensor_add(z[:], z[:], biaB[:])
                    nc.vector.tensor_add(z[:], z[:], xo_t[:])
                    zm = phc.tile([P, F], f32, name="zm")
                    nc.vector.tensor_scalar_min(zm[:], z[:], 0.0)
                    nc.scalar.activation(out=zm[:], in_=zm[:], func=AF.Exp)
                    nc.vector.tensor_scalar_max(z[:], z[:], 0.0)
                    nc.vector.tensor_add(z[:], z[:], zm[:])
                    nc.vector.tensor_scalar_add(z[:], z[:], -1.0)
                    nc.sync.dma_start(out=out_d[t * P:(t + 1) * P, :], in_=z[:])
    if finalize:
        nc.finalize()
    return nc


# ---------------------------------------------------------------- driver
def _run_gat(x, edge_index, edge_attr, W, att_src, att_dst, W_e, att_edge,
             gamma, beta, cfg, trace=False, return_results=False):
    from concourse.bass_utils import run_bass_kernel_spmd

    N, F, H, Dh, NC = cfg["N"], cfg["F"], cfg["H"], cfg["Dh"], cfg["NC"]
    Np = N // NC
    FX = 192
    Wb, v4 = _fold_weights(
        np.asarray(W, np.float32), np.asarray(att_src, np.float32),
        np.asarray(att_dst, np.float32), np.asarray(W_e, np.float32),
        np.asarray(att_edge, np.float32), H, Dh, FX)
    cores, meta = _preprocess(edge_index, edge_attr, cfg)
    nc = _build(cfg, meta)

    x_np = np.asarray(x, np.float32)
    xT = np.ascontiguousarray(x_np.T)
    gam = np.asarray(gamma, np.float32)
    bet = np.asarray(beta, np.float32)
    n_tiles = len(meta["CAs"])
    in_maps = []
    for c in range(NC):
        st = cores[c]["in"]
        order = cores[c]["node_order"]
        gl = np.where(order >= 0, c * Np + order, 0)
        xo = x_np[gl]
        xo[order < 0] = 0.0
        xTo = np.ascontiguousarray(xo.T)
        in_maps.append(dict(
            xT=xT, xTo=xTo, xo=np.ascontiguousarray(xo), Wb=Wb, v4=v4,
            idxA=st["idxA"], idxB=st["idxB"],
            eaT4=st["eaT4"], rdeg=st["rdeg"], gamma=gam, beta=bet))
    res = run_bass_kernel_spmd(nc, in_maps, core_ids=list(range(NC)),
                               trace=trace)
    out = np.empty((N, F), dtype=np.float32)
    for c in range(NC):
        oc = res.results[c]["out"]
        order = cores[c]["node_order"]
        real = order >= 0
        out[c * Np + order[real]] = oc[np.nonzero(real)[0]]
    if return_results:
        return out, res
    return out


def kernel(x, edge_index, edge_attr, W, att_src, att_dst, W_e, att_edge,
           gamma, beta):
    return _run_gat(x, edge_index, edge_attr, W, att_src, att_dst, W_e,
                    att_edge, gamma, beta, _cfg_full())



# revision 15
# speedup vs baseline: 3.6438x; 3.6438x over previous
"""GAT block (GATConv + InstanceNorm + residual + ELU) on 8 Trainium2 NeuronCores.

Strategy (graph/data parallel over dst nodes), gather-free:
  - Host routes each edge to the core owning its dst node; per core, dst
    nodes are sorted by degree and grouped into tiles of 128 (dst node ==
    partition, so aggregation is a free-dim reduce).
  - Incoming edges of a tile live in padded slot columns k=1..C-1 (k=0 is
    the self loop, filled from on-chip hx_own). Slot rows are NOT gathered:
    the host lays out raw x[src] feature rows in slot order (xsT, bf16,
    feature-major), and the device computes h = xs @ Wb per slot column
    with one TensorE matmul each (lhsT = 128x128 xsT column chunk,
    rhs = Wb [F, F+2H]) -- dense DMA + PE replaces dma_gather entirely.
  - Pad slots have zero x rows; a host-built mask [P, C] of 0/-1e30 is
    added to the logits so softmax kills them. Softmax skips the segment
    max (logits bounded; exp clamped at -88).
  - a_edge = edge_attr @ v (v folded on host) via TensorE on a
    host-transposed 4-slot-interleaved eaT4 layout; the self loop's a_edge
    is (sum_k a_edge_k) / deg (linearity in edge_attr).
  - InstanceNorm stats via ones-matmul partition reduction, AllReduce'd
    across the 8 cores; finalize = per-channel affine + residual + ELU.
"""

import math
import numpy as np

P = 128


def _cfg_full():
    return dict(N=50000, E=1600000, F=128, H=8, Dh=16, ED=16, NC=8)


def _fold_weights(W, att_src, att_dst, W_e, att_edge, H, Dh):
    F = W.shape[0]
    FX = F + 2 * H
    w_src = np.stack(
        [W[:, h * Dh:(h + 1) * Dh] @ att_src[h] for h in range(H)], axis=1)
    w_dst = np.stack(
        [W[:, h * Dh:(h + 1) * Dh] @ att_dst[h] for h in range(H)], axis=1)
    Wb = np.zeros((F, FX), dtype=np.float32)
    Wb[:, :F] = W
    Wb[:, F:F + H] = w_src
    Wb[:, F + H:F + 2 * H] = w_dst
    v = np.stack(
        [W_e[:, h * Dh:(h + 1) * Dh] @ att_edge[h] for h in range(H)], axis=1
    ).astype(np.float32)
    ED = W_e.shape[0]
    v4 = np.zeros((4 * ED, 4 * H), dtype=np.float32)
    for j in range(4):
        v4[j * ED:(j + 1) * ED, j * H:(j + 1) * H] = v
    return Wb, v4


def _preprocess(x, edge_index, edge_attr, cfg):
    N, F, ED, NC = cfg["N"], cfg["F"], cfg["ED"], cfg["NC"]
    Np = N // NC
    n_tiles = math.ceil(Np / P)
    src = np.asarray(edge_index[0]).astype(np.int64)
    dst = np.asarray(edge_index[1]).astype(np.int64)
    ea = np.asarray(edge_attr, dtype=np.float32)
    x_np = np.asarray(x, dtype=np.float32)

    cores = []
    for c in range(NC):
        m = (dst >= c * Np) & (dst < (c + 1) * Np)
        e_ids = np.nonzero(m)[0]
        dst_c = dst[e_ids] - c * Np
        order_e = np.argsort(dst_c, kind="stable")
        e_ids = e_ids[order_e]
        dst_c = dst_c[order_e]
        deg = np.bincount(dst_c, minlength=Np).astype(np.int64)
        cum = np.zeros(Np + 1, dtype=np.int64)
        np.cumsum(deg, out=cum[1:])
        node_order = np.argsort(-deg, kind="stable")
        pad_nodes = n_tiles * P - Np
        node_order_p = np.concatenate(
            [node_order, np.full(pad_nodes, -1, dtype=np.int64)])
        Ks = []
        for t in range(n_tiles):
            nt = node_order_p[t * P:(t + 1) * P]
            real = nt[nt >= 0]
            Ks.append(int(deg[real].max()) if len(real) else 0)
        cores.append(dict(e_ids=e_ids, dst_c=dst_c, deg=deg, cum=cum,
                          node_order=node_order_p, Ks=Ks))

    # C_t = 1 self col + max_deg, padded to %4 (quad eaT4), common across cores
    Cs = []
    for t in range(n_tiles):
        k = max(c["Ks"][t] for c in cores)
        Cs.append(max(((1 + k + 3) // 4) * 4, 4))
    Carr = np.array(Cs, dtype=np.int64)
    offX = np.zeros(n_tiles + 1, dtype=np.int64)
    np.cumsum((Carr - 1) * P, out=offX[1:])       # xsT slot columns (k>=1)
    offs4 = np.zeros(n_tiles + 1, dtype=np.int64)  # eaT4 quad-column offsets
    np.cumsum(Carr // 4 * P, out=offs4[1:])
    offC = np.zeros(n_tiles + 1, dtype=np.int64)   # mask columns
    np.cumsum(Carr, out=offC[1:])
    SX, S4, SC = int(offX[-1]), int(offs4[-1]), int(offC[-1])

    for c in range(NC):
        st = cores[c]
        deg, cum = st["deg"], st["cum"]
        node_order = st["node_order"]
        rdeg = np.ones(n_tiles * P, dtype=np.float32)
        tile_of_pos = np.repeat(np.arange(n_tiles), P)
        p_of_pos = np.tile(np.arange(P), n_tiles)
        real_m = node_order >= 0
        nodes = node_order[real_m]
        rdeg[real_m] = 1.0 / np.maximum(deg[nodes], 1).astype(np.float32)
        pos_r = np.nonzero(real_m)[0]
        pos_of_node = np.empty(Np, dtype=np.int64)
        pos_of_node[nodes] = pos_r
        nloc = st["dst_c"]
        e_pos = pos_of_node[nloc]
        e_t = tile_of_pos[e_pos]
        e_p = p_of_pos[e_pos]
        k_e = np.arange(len(nloc)) - cum[nloc] + 1   # slot col 1..deg
        e_srcs = src[st["e_ids"]]

        # xsT: raw x rows of each slot, feature-major bf16 (zeros = pad)
        xs = np.zeros((SX, F), dtype=np.float32)
        jX = offX[e_t] + (k_e - 1) * P + e_p
        xs[jX] = x_np[e_srcs]
        st_xsT = np.ascontiguousarray(xs.T)

        # mask [P, SC]: 0 for self col + real slots, -1e30 for pad
        mask = np.full((P, SC), -1e30, dtype=np.float32)
        mask[:, offC[:-1]] = 0.0                      # self col of each tile
        mask[e_p, offC[e_t] + k_e] = 0.0

        # eaT4: slot k -> quad col k>>2, sub-slot k&3 (k=0 self stays zero)
        eaT4 = np.zeros((4 * ED, S4), dtype=np.float32)
        col = offs4[e_t] + (k_e >> 2) * P + e_p
        jj = (k_e & 3).astype(np.int64)
        ea_c = ea[st["e_ids"]]
        for j4 in range(4):
            mj = jj == j4
            eaT4[j4 * ED:(j4 + 1) * ED, col[mj]] = ea_c[mj].T
        st["in"] = dict(xsT=st_xsT, eaT4=eaT4, rdeg=rdeg, mask=mask)
    return cores, dict(Cs=Cs, offX=offX, offs4=offs4, offC=offC)


# ---------------------------------------------------------------- device
def _build(cfg, meta, finalize=True):
    import concourse.bass as bass
    import concourse.bacc as bacc
    import concourse.tile as tile
    from concourse import mybir

    N, F, H, ED, NC = cfg["N"], cfg["F"], cfg["H"], cfg["ED"], cfg["NC"]
    Np = N // NC
    Cs = meta["Cs"]
    offX, offs4, offC = meta["offX"], meta["offs4"], meta["offC"]
    n_tiles = len(Cs)
    FU = F + 2 * H           # Wb output columns
    SX, S4, SC = int(offX[-1]), int(offs4[-1]), int(offC[-1])
    f32 = mybir.dt.float32
    bf16 = mybir.dt.bfloat16
    AF = mybir.ActivationFunctionType
    OP = mybir.AluOpType
    EPS_IN, NEG = 1e-5, 0.2

    nc = bacc.Bacc("TRN2", target_bir_lowering=False, debug=False,
                   num_devices=NC)
    xsT_d = nc.declare_dram_parameter("xsT", [F, SX], bf16, isOutput=False)
    xsTo_d = nc.declare_dram_parameter("xsTo", [F, n_tiles * P], bf16,
                                       isOutput=False)
    xo_d = nc.declare_dram_parameter("xo", [n_tiles * P, F], f32,
                                     isOutput=False)
    Wb_d = nc.declare_dram_parameter("Wb", [F, FU], bf16, isOutput=False)
    v4_d = nc.declare_dram_parameter("v4", [4 * ED, 4 * H], f32, isOutput=False)
    ea4_d = nc.declare_dram_parameter("eaT4", [4 * ED, S4], f32,
                                      isOutput=False)
    mask_d = nc.declare_dram_parameter("mask", [P, SC], f32, isOutput=False)
    rdeg_d = nc.declare_dram_parameter("rdeg", [n_tiles * P], f32,
                                       isOutput=False)
    gam_d = nc.declare_dram_parameter("gamma", [F], f32, isOutput=False)
    bet_d = nc.declare_dram_parameter("beta", [F], f32, isOutput=False)
    out_d = nc.declare_dram_parameter("out", [n_tiles * P, F], f32,
                                      isOutput=True)

    with tile.TileContext(nc) as tc:
        with (
            tc.tile_pool(name="dram", bufs=1, space="DRAM") as dram,
            tc.tile_pool(name="consts", bufs=1) as consts,
            tc.tile_pool(name="ph_a", bufs=3) as pha,
            tc.tile_pool(name="h_ps", bufs=4, space="PSUM") as h_ps,
            tc.tile_pool(name="ph_b", bufs=3) as phb,
            tc.tile_pool(name="ph_b_ps", bufs=2, space="PSUM") as phb_ps,
            tc.tile_pool(name="stats_ps", bufs=2, space="PSUM") as stats_ps,
            tc.tile_pool(name="keep", bufs=1) as keep,
        ):
            Wb_s = consts.tile([F, FU], bf16)
            nc.sync.dma_start(out=Wb_s[:], in_=Wb_d[:, :])
            v4_s = consts.tile([4 * ED, 4 * H], f32)
            nc.sync.dma_start(out=v4_s[:], in_=v4_d[:, :])
            ones = consts.tile([P, 1], f32)
            nc.vector.memset(ones[:], 1.0)

            # hx_own: own nodes in tile order (self-loop slot, a_dst)
            KC = 3  # slot columns per PSUM bank (3*FU f32 <= 2KB)
            hx_own = keep.tile([P, n_tiles, FU], f32)
            for t in range(n_tiles):
                xTo_t = pha.tile([F, P], bf16, name="xTo_t")
                nc.sync.dma_start(out=xTo_t[:], in_=xsTo_d[:, t * P:(t + 1) * P])
                ho_p = h_ps.tile([P, KC * FU], f32, name="ho_p", tag="hp")
                nc.tensor.matmul(out=ho_p[:, :FU], lhsT=xTo_t[:], rhs=Wb_s[:],
                                 start=True, stop=True)
                nc.vector.tensor_copy(out=hx_own[:, t, :], in_=ho_p[:, :FU])

            # ---------------- Phase B: per-tile h, attention, aggregation
            out_all = keep.tile([P, n_tiles, F], f32)
            acc = keep.tile([P, 2], f32)
            nc.vector.memset(acc[:], 0.0)

            for t in range(n_tiles):
                C = Cs[t]
                C4 = C // 4

                xs_t = phb.tile([F, (C - 1) * P], bf16, name="xs_t", tag="xs_t")
                nc.sync.dma_start(out=xs_t[:],
                                  in_=xsT_d[:, int(offX[t]):int(offX[t + 1])])
                ea4_t = phb.tile([4 * ED, C4 * P], f32, name="ea4_t",
                                 tag="ea4_t")
                nc.sync.dma_start(
                    out=ea4_t[:],
                    in_=ea4_d[:, int(offs4[t]):int(offs4[t + 1])])
                mask_t = phb.tile([P, C], f32, name="mask_t", tag="mask_t")
                nc.sync.dma_start(out=mask_t[:],
                                  in_=mask_d[:, int(offC[t]):int(offC[t + 1])])
                rdeg_t = phb.tile([P, 1], f32, name="rdeg_t", tag="rdeg_t")
                nc.sync.dma_start(out=rdeg_t[:],
                                  in_=rdeg_d[t * P:(t + 1) * P, None])

                # slot rows: h = xs @ Wb per column, 3 columns per PSUM bank
                g = phb.tile([P, C, FU], bf16, name="g", tag="g")
                nc.vector.tensor_copy(out=g[:, 0, :], in_=hx_own[:, t, :])
                for k0 in range(1, C, KC):
                    nk = min(KC, C - k0)
                    hp = h_ps.tile([P, KC * FU], f32, name="hp", tag="hp")
                    for i in range(nk):
                        nc.tensor.matmul(
                            out=hp[:, i * FU:(i + 1) * FU],
                            lhsT=xs_t[:, (k0 - 1 + i) * P:(k0 + i) * P],
                            rhs=Wb_s[:], start=True, stop=True)
                    nc.vector.tensor_copy(
                        out=g[:, k0:k0 + nk, :],
                        in_=hp[:, :nk * FU].rearrange("p (k f) -> p k f",
                                                      k=nk))

                # a_edge: quad matmuls [4ED,P] @ [4ED,4H]
                ae = phb.tile([P, C, H], f32, name="ae", tag="ae")
                QG = 16
                for qg in range(math.ceil(C4 / QG)):
                    nq = min(QG, C4 - qg * QG)
                    ae_p = phb_ps.tile([P, QG * 4 * H], f32, name="ae_p",
                                       tag="ae_p")
                    for qi in range(nq):
                        q = qg * QG + qi
                        nc.tensor.matmul(
                            out=ae_p[:, qi * 4 * H:(qi + 1) * 4 * H],
                            lhsT=ea4_t[:, q * P:(q + 1) * P],
                            rhs=v4_s[:],
                            start=True, stop=True)
                    nc.vector.tensor_copy(
                        out=ae[:, qg * QG * 4:qg * QG * 4 + nq * 4, :],
                        in_=ae_p[:, :nq * 4 * H])
                # self-loop a_edge = (sum of a_edge over all slots) / deg
                aeL = phb.tile([P, H], f32, name="aeL", tag="aeL")
                nc.vector.tensor_reduce(
                    out=aeL[:], in_=ae.transpose([0, 2, 1]),
                    axis=mybir.AxisListType.X, op=OP.add)
                nc.vector.tensor_scalar_mul(ae[:, 0, :], aeL[:], rdeg_t[:])

                # logits -> exp(leaky) ; no segment max (clamped at -88)
                asl = phb.tile([P, C, H], f32, name="asl", tag="asl")
                nc.vector.tensor_copy(out=asl[:], in_=g[:, :, F:F + H])
                al = phb.tile([P, H, C], f32, name="al", tag="al")
                alv = al.transpose([0, 2, 1])
                nc.vector.tensor_tensor(
                    out=alv, in0=asl[:], in1=ae[:, :, :], op=OP.add)
                adst = hx_own[:, t, F + H:F + 2 * H]
                nc.vector.tensor_tensor(
                    out=alv, in0=alv,
                    in1=adst.unsqueeze(1).broadcast_to((P, C, H)),
                    op=OP.add)
                nc.vector.tensor_tensor(
                    out=alv, in0=alv,
                    in1=mask_t.unsqueeze(2).broadcast_to((P, C, H)),
                    op=OP.add)
                tl = phb.tile([P, H, C], f32, name="tl", tag="tl")
                nc.vector.tensor_scalar_mul(tl[:], al[:], NEG)
                nc.vector.tensor_tensor(out=al[:], in0=al[:], in1=tl[:],
                                        op=OP.max)
                nc.vector.tensor_scalar_max(al[:], al[:], -88.0)
                nc.scalar.activation(out=al[:], in_=al[:], func=AF.Exp)
                al16 = phb.tile([P, H, C], bf16, name="al16", tag="al16")
                nc.vector.tensor_copy(out=al16[:], in_=al[:])

                den = phb.tile([P, H], f32, name="den", tag="den")
                nc.vector.tensor_reduce(
                    out=den[:], in_=al[:],
                    axis=mybir.AxisListType.X, op=OP.add)
                gh = g[:, :, 0:F].rearrange("p k (h d) -> p k h d", h=H)
                nc.vector.tensor_tensor(
                    out=gh, in0=gh,
                    in1=al16.transpose([0, 2, 1]).unsqueeze(3)
                        .broadcast_to((P, C, H, F // H)),
                    op=OP.mult)
                msg = phb.tile([P, F], f32, name="msg", tag="msg")
                nc.vector.tensor_reduce(
                    out=msg[:],
                    in_=g[:, :, 0:F].transpose([0, 2, 1]),
                    axis=mybir.AxisListType.X, op=OP.add)

                # out_pre = msg / den  (per-node alpha normalization)
                rec = phb.tile([P, H], f32, name="rec", tag="rec")
                nc.vector.tensor_scalar_add(rec[:], den[:], 1e-16)
                nc.vector.reciprocal(rec[:], rec[:])
                op_t = out_all[:, t, :]
                nc.vector.tensor_tensor(
                    out=op_t.rearrange("p (h d) -> p h d", h=H),
                    in0=msg.rearrange("p (h d) -> p h d", h=H),
                    in1=rec.unsqueeze(2).broadcast_to((P, H, F // H)),
                    op=OP.mult)

                # stats: per-channel sum & sumsq via ones-matmul
                sq = phb.tile([P, F], f32, name="sq", tag="sq")
                nc.vector.tensor_mul(sq[:], op_t, op_t)
                st_p = stats_ps.tile([P, 2], f32, name="st_p", tag="st_p")
                nc.tensor.matmul(out=st_p[:, 0:1], lhsT=op_t, rhs=ones[:],
                                 start=True, stop=True)
                nc.tensor.matmul(out=st_p[:, 1:2], lhsT=sq[:], rhs=ones[:],
                                 start=True, stop=True)
                nc.vector.tensor_add(acc[:], acc[:], st_p[:])

            # ---------------- Phase C: stats allreduce + normalize + ELU
            st_in = dram.tile([P, 2], f32)
            st_out = dram.tile([P, 2], f32, addr_space="Shared")
            nc.sync.dma_start(out=st_in[:], in_=acc[:])
            nc.gpsimd.collective_compute(
                "AllReduce", OP.add,
                replica_groups=[list(range(NC))],
                ins=[st_in[:].opt()], outs=[st_out[:].opt()])
            sg = keep.tile([P, 2], f32)
            nc.sync.dma_start(out=sg[:], in_=st_out[:])
            mean = keep.tile([P, 1], f32)
            nc.vector.tensor_scalar_mul(mean[:], sg[:, 0:1], 1.0 / N)
            ex2 = keep.tile([P, 1], f32)
            nc.vector.tensor_scalar_mul(ex2[:], sg[:, 1:2], 1.0 / N)
            var = keep.tile([P, 1], f32)
            nc.vector.tensor_mul(var[:], mean[:], mean[:])
            nc.vector.tensor_sub(var[:], ex2[:], var[:])
            rstd = keep.tile([P, 1], f32)
            eps_t = keep.tile([P, 1], f32)
            nc.vector.memset(eps_t[:], EPS_IN)
            nc.scalar.activation(out=rstd[:], in_=var[:], func=AF.Sqrt,
                                 bias=eps_t[:])
            nc.vector.reciprocal(rstd[:], rstd[:])
            gam_s = keep.tile([P, 1], f32)
            nc.sync.dma_start(out=gam_s[:], in_=gam_d[:, None])
            bet_s = keep.tile([P, 1], f32)
            nc.sync.dma_start(out=bet_s[:], in_=bet_d[:, None])
            scl = keep.tile([P, 1], f32)
            nc.vector.tensor_mul(scl[:], rstd[:], gam_s[:])
            bia = keep.tile([P, 1], f32)
            nc.vector.tensor_mul(bia[:], mean[:], scl[:])
            nc.vector.tensor_sub(bia[:], bet_s[:], bia[:])
            sb_dram = dram.tile([2, P], f32)
            nc.sync.dma_start(out=sb_dram[0, :], in_=scl[:, 0])
            nc.sync.dma_start(out=sb_dram[1, :], in_=bia[:, 0])
            sclB = keep.tile([P, F], f32)
            nc.sync.dma_start(out=sclB[:],
                              in_=sb_dram[0:1, :].broadcast_to((P, P)))
            biaB = keep.tile([P, F], f32)
            nc.sync.dma_start(out=biaB[:],
                              in_=sb_dram[1:2, :].broadcast_to((P, P)))

            with tc.tile_pool(name="ph_c", bufs=3) as phc:
                for t in range(n_tiles):
                    xo_t = phc.tile([P, F], f32, name="xo_t")
                    nc.sync.dma_start(out=xo_t[:],
                                      in_=xo_d[t * P:(t + 1) * P, :])
                    z = phc.tile([P, F], f32, name="z")
                    nc.vector.tensor_mul(z[:], out_all[:, t, :], sclB[:])
                    nc.vector.tensor_add(z[:], z[:], biaB[:])
                    nc.vector.tensor_add(z[:], z[:], xo_t[:])
                    zm = phc.tile([P, F], f32, name="zm")
                    nc.vector.tensor_scalar_min(zm[:], z[:], 0.0)
                    nc.scalar.activation(out=zm[:], in_=zm[:], func=AF.Exp)
                    nc.vector.tensor_scalar_max(z[:], z[:], 0.0)
                    nc.vector.tensor_add(z[:], z[:], zm[:])
                    nc.vector.tensor_scalar_add(z[:], z[:], -1.0)
                    nc.sync.dma_start(out=out_d[t * P:(t + 1) * P, :], in_=z[:])
    if finalize:
        nc.finalize()
    return nc


# ---------------------------------------------------------------- driver
def _to_bf16(a):
    import ml_dtypes
    return a.astype(ml_dtypes.bfloat16)


def _run_gat(x, edge_index, edge_attr, W, att_src, att_dst, W_e, att_edge,
             gamma, beta, cfg, trace=False, return_results=False):
    from concourse.bass_utils import run_bass_kernel_spmd

    N, F, H, Dh, NC = cfg["N"], cfg["F"], cfg["H"], cfg["Dh"], cfg["NC"]
    Np = N // NC
    Wb, v4 = _fold_weights(
        np.asarray(W, np.float32), np.asarray(att_src, np.float32),
        np.asarray(att_dst, np.float32), np.asarray(W_e, np.float32),
        np.asarray(att_edge, np.float32), H, Dh)
    x_np = np.asarray(x, np.float32)
    cores, meta = _preprocess(x_np, edge_index, edge_attr, cfg)
    nc = _build(cfg, meta)

    gam = np.asarray(gamma, np.float32)
    bet = np.asarray(beta, np.float32)
    n_tiles = len(meta["Cs"])
    Wb16 = _to_bf16(Wb)
    in_maps = []
    for c in range(NC):
        st = cores[c]["in"]
        order = cores[c]["node_order"]
        gl = np.where(order >= 0, c * Np + order, 0)
        xo = x_np[gl]
        xo[order < 0] = 0.0
        xsTo = _to_bf16(np.ascontiguousarray(xo.T))
        in_maps.append(dict(
            xsT=_to_bf16(st["xsT"]), xsTo=xsTo,
            xo=np.ascontiguousarray(xo), Wb=Wb16, v4=v4,
            eaT4=st["eaT4"], mask=st["mask"], rdeg=st["rdeg"],
            gamma=gam, beta=bet))
    res = run_bass_kernel_spmd(nc, in_maps, core_ids=list(range(NC)),
                               trace=trace)
    out = np.empty((N, F), dtype=np.float32)
    for c in range(NC):
        oc = res.results[c]["out"]
        order = cores[c]["node_order"]
        real = order >= 0
        out[c * Np + order[real]] = oc[np.nonzero(real)[0]]
    if return_results:
        return out, res
    return out


def kernel(x, edge_index, edge_attr, W, att_src, att_dst, W_e, att_edge,
           gamma, beta):
    return _run_gat(x, edge_index, edge_attr, W, att_src, att_dst, W_e,
                    att_edge, gamma, beta, _cfg_full())


# revision 37
# speedup vs baseline: 4.5083x; 1.2373x over previous
"""GAT block (GATConv + InstanceNorm + residual + ELU) on 8 Trainium2 NeuronCores.

Strategy (graph/data parallel over dst nodes), gather-free:
  - Host routes each edge to the core owning its dst node; per core, dst
    nodes are sorted by degree and grouped into tiles of 128 (dst node ==
    partition, so aggregation is a free-dim reduce).
  - Incoming edges of a tile live in padded slot columns k=1..C-1 (k=0 is
    the self loop, filled from on-chip hx_own). Slot rows are NOT gathered:
    the host lays out raw x[src] feature rows in slot order (xsT, bf16,
    feature-major), and the device computes h = xs @ Wb per slot column
    with one TensorE matmul each (lhsT = 128x128 xsT column chunk,
    rhs = Wb [F, F+2H]) -- dense DMA + PE replaces dma_gather entirely.
  - Pad slots have zero x rows; a host-built mask [P, C] of 0/-1e30 is
    added to the logits so softmax kills them. Softmax skips the segment
    max (logits bounded; exp clamped at -88).
  - a_edge = edge_attr @ v (v folded on host) via TensorE on a
    host-transposed 4-slot-interleaved eaT4 layout; the self loop's a_edge
    is (sum_k a_edge_k) / deg (linearity in edge_attr).
  - InstanceNorm stats via ones-matmul partition reduction, AllReduce'd
    across the 8 cores; finalize = per-channel affine + residual + ELU.
"""

import math
import numpy as np

P = 128


def _cfg_full():
    return dict(N=50000, E=1600000, F=128, H=8, Dh=16, ED=16, NC=8)


def _fold_weights(W, att_src, att_dst, W_e, att_edge, H, Dh):
    F = W.shape[0]
    FX = F + 2 * H
    w_src = np.stack(
        [W[:, h * Dh:(h + 1) * Dh] @ att_src[h] for h in range(H)], axis=1)
    w_dst = np.stack(
        [W[:, h * Dh:(h + 1) * Dh] @ att_dst[h] for h in range(H)], axis=1)
    Wb = np.zeros((F, FX), dtype=np.float32)
    Wb[:, :F] = W
    Wb[:, F:F + H] = w_src
    Wb[:, F + H:F + 2 * H] = w_dst
    v = np.stack(
        [W_e[:, h * Dh:(h + 1) * Dh] @ att_edge[h] for h in range(H)], axis=1
    ).astype(np.float32)
    ED = W_e.shape[0]
    v4 = np.zeros((4 * ED, 4 * H), dtype=np.float32)
    for j in range(4):
        v4[j * ED:(j + 1) * ED, j * H:(j + 1) * H] = v
    return Wb, v4


def _preprocess(x, edge_index, edge_attr, cfg):
    N, F, ED, NC = cfg["N"], cfg["F"], cfg["ED"], cfg["NC"]
    Np = N // NC
    n_tiles = math.ceil(Np / P)
    src = np.asarray(edge_index[0]).astype(np.int64)
    dst = np.asarray(edge_index[1]).astype(np.int64)
    ea = np.asarray(edge_attr, dtype=np.float32)
    x_np = np.asarray(x, dtype=np.float32)

    cores = []
    for c in range(NC):
        m = (dst >= c * Np) & (dst < (c + 1) * Np)
        e_ids = np.nonzero(m)[0]
        dst_c = dst[e_ids] - c * Np
        order_e = np.argsort(dst_c, kind="stable")
        e_ids = e_ids[order_e]
        dst_c = dst_c[order_e]
        deg = np.bincount(dst_c, minlength=Np).astype(np.int64)
        cum = np.zeros(Np + 1, dtype=np.int64)
        np.cumsum(deg, out=cum[1:])
        node_order = np.argsort(-deg, kind="stable")
        pad_nodes = n_tiles * P - Np
        node_order_p = np.concatenate(
            [node_order, np.full(pad_nodes, -1, dtype=np.int64)])
        Ks = []
        for t in range(n_tiles):
            nt = node_order_p[t * P:(t + 1) * P]
            real = nt[nt >= 0]
            Ks.append(int(deg[real].max()) if len(real) else 0)
        cores.append(dict(e_ids=e_ids, dst_c=dst_c, deg=deg, cum=cum,
                          node_order=node_order_p, Ks=Ks))

    # C_t = 1 self col + max_deg, padded to %4 (quad eaT4), common across cores
    Cs = []
    for t in range(n_tiles):
        k = max(c["Ks"][t] for c in cores)
        Cs.append(max(((1 + k + 3) // 4) * 4, 4))
    Carr = np.array(Cs, dtype=np.int64)
    offX = np.zeros(n_tiles + 1, dtype=np.int64)
    np.cumsum((Carr - 1) * P, out=offX[1:])       # xsT slot columns (k>=1)
    offs4 = np.zeros(n_tiles + 1, dtype=np.int64)  # eaT4 quad-column offsets
    np.cumsum(Carr // 4 * P, out=offs4[1:])
    offC = np.zeros(n_tiles + 1, dtype=np.int64)   # mask columns
    np.cumsum(Carr, out=offC[1:])
    SX, S4, SC = int(offX[-1]), int(offs4[-1]), int(offC[-1])

    for c in range(NC):
        st = cores[c]
        deg, cum = st["deg"], st["cum"]
        node_order = st["node_order"]
        rdeg = np.ones(n_tiles * P, dtype=np.float32)
        tile_of_pos = np.repeat(np.arange(n_tiles), P)
        p_of_pos = np.tile(np.arange(P), n_tiles)
        real_m = node_order >= 0
        nodes = node_order[real_m]
        rdeg[real_m] = 1.0 / np.maximum(deg[nodes], 1).astype(np.float32)
        pos_r = np.nonzero(real_m)[0]
        pos_of_node = np.empty(Np, dtype=np.int64)
        pos_of_node[nodes] = pos_r
        nloc = st["dst_c"]
        e_pos = pos_of_node[nloc]
        e_t = tile_of_pos[e_pos]
        e_p = p_of_pos[e_pos]
        k_e = np.arange(len(nloc)) - cum[nloc] + 1   # slot col 1..deg
        e_srcs = src[st["e_ids"]]

        # xsT: raw x rows of each slot, feature-major bf16 (zeros = pad)
        xs = np.zeros((SX, F), dtype=np.float32)
        jX = offX[e_t] + (k_e - 1) * P + e_p
        xs[jX] = x_np[e_srcs]
        st_xsT = np.ascontiguousarray(xs.T)

        # mask [P, SC]: 0 for self col + real slots, -1e30 for pad
        mask = np.full((P, SC), -1e30, dtype=np.float32)
        mask[:, offC[:-1]] = 0.0                      # self col of each tile
        mask[e_p, offC[e_t] + k_e] = 0.0

        # eaT4: slot k -> quad col k>>2, sub-slot k&3 (k=0 self stays zero)
        eaT4 = np.zeros((4 * ED, S4), dtype=np.float32)
        col = offs4[e_t] + (k_e >> 2) * P + e_p
        jj = (k_e & 3).astype(np.int64)
        ea_c = ea[st["e_ids"]]
        for j4 in range(4):
            mj = jj == j4
            eaT4[j4 * ED:(j4 + 1) * ED, col[mj]] = ea_c[mj].T
        st["in"] = dict(xsT=st_xsT, eaT4=eaT4, rdeg=rdeg, mask=mask)
    return cores, dict(Cs=Cs, offX=offX, offs4=offs4, offC=offC)


# ---------------------------------------------------------------- device
def _build(cfg, meta, finalize=True):
    import concourse.bass as bass
    import concourse.bacc as bacc
    import concourse.tile as tile
    from concourse import mybir

    N, F, H, ED, NC = cfg["N"], cfg["F"], cfg["H"], cfg["ED"], cfg["NC"]
    Np = N // NC
    Cs = meta["Cs"]
    offX, offs4, offC = meta["offX"], meta["offs4"], meta["offC"]
    n_tiles = len(Cs)
    FU = F + 2 * H           # Wb output columns
    SX, S4, SC = int(offX[-1]), int(offs4[-1]), int(offC[-1])
    f32 = mybir.dt.float32
    bf16 = mybir.dt.bfloat16
    AF = mybir.ActivationFunctionType
    OP = mybir.AluOpType
    EPS_IN, NEG = 1e-5, 0.2

    nc = bacc.Bacc("TRN2", target_bir_lowering=False, debug=False,
                   num_devices=NC)
    xsT_d = nc.declare_dram_parameter("xsT", [F, SX], bf16, isOutput=False)
    xsTo_d = nc.declare_dram_parameter("xsTo", [F, n_tiles * P], bf16,
                                       isOutput=False)
    xo_d = nc.declare_dram_parameter("xo", [n_tiles * P, F], f32,
                                     isOutput=False)
    Wb_d = nc.declare_dram_parameter("Wb", [F, FU], bf16, isOutput=False)
    v4_d = nc.declare_dram_parameter("v4", [4 * ED, 4 * H], f32, isOutput=False)
    ea4_d = nc.declare_dram_parameter("eaT4", [4 * ED, S4], f32,
                                      isOutput=False)
    mask_d = nc.declare_dram_parameter("mask", [P, SC], f32, isOutput=False)
    rdeg_d = nc.declare_dram_parameter("rdeg", [n_tiles * P], f32,
                                       isOutput=False)
    gam_d = nc.declare_dram_parameter("gamma", [F], f32, isOutput=False)
    bet_d = nc.declare_dram_parameter("beta", [F], f32, isOutput=False)
    out_d = nc.declare_dram_parameter("out", [n_tiles * P, F], f32,
                                      isOutput=True)

    with tile.TileContext(nc) as tc:
        with (
            tc.tile_pool(name="dram", bufs=1, space="DRAM") as dram,
            tc.tile_pool(name="consts", bufs=1) as consts,
            tc.tile_pool(name="ph_a", bufs=3) as pha,
            tc.tile_pool(name="h_ps", bufs=4, space="PSUM") as h_ps,
            tc.tile_pool(name="ph_b", bufs=3) as phb,
            tc.tile_pool(name="ph_b_ps", bufs=2, space="PSUM") as phb_ps,
            tc.tile_pool(name="stats_ps", bufs=2, space="PSUM") as stats_ps,
            tc.tile_pool(name="keep", bufs=1) as keep,
        ):
            Wb_s = consts.tile([F, FU], bf16)
            nc.sync.dma_start(out=Wb_s[:], in_=Wb_d[:, :])
            v4_s = consts.tile([4 * ED, 4 * H], f32)
            nc.sync.dma_start(out=v4_s[:], in_=v4_d[:, :])
            ones = consts.tile([P, 1], f32)
            nc.vector.memset(ones[:], 1.0)

            # hx_own: own nodes in tile order (self-loop slot, a_dst)
            KC = 3  # slot columns per PSUM bank (3*FU f32 <= 2KB)
            hx_own = keep.tile([P, n_tiles, FU], f32)
            for t in range(n_tiles):
                xTo_t = pha.tile([F, P], bf16, name="xTo_t")
                nc.sync.dma_start(out=xTo_t[:], in_=xsTo_d[:, t * P:(t + 1) * P])
                ho_p = h_ps.tile([P, KC * FU], f32, name="ho_p", tag="hp")
                nc.tensor.matmul(out=ho_p[:, :FU], lhsT=xTo_t[:], rhs=Wb_s[:],
                                 start=True, stop=True)
                nc.scalar.copy(out=hx_own[:, t, :], in_=ho_p[:, :FU])

            # ---------------- Phase B: per-tile h, attention, aggregation
            out_all = keep.tile([P, n_tiles, F], f32)
            acc = keep.tile([P, 2], f32)
            nc.vector.memset(acc[:], 0.0)

            for t in range(n_tiles):
                C = Cs[t]
                C4 = C // 4

                xs_t = phb.tile([F, (C - 1) * P], bf16, name="xs_t", tag="xs_t")
                nc.sync.dma_start(out=xs_t[:],
                                  in_=xsT_d[:, int(offX[t]):int(offX[t + 1])])
                ea4_t = phb.tile([4 * ED, C4 * P], f32, name="ea4_t",
                                 tag="ea4_t")
                nc.sync.dma_start(
                    out=ea4_t[:],
                    in_=ea4_d[:, int(offs4[t]):int(offs4[t + 1])])
                mask_t = phb.tile([P, C], f32, name="mask_t", tag="mask_t")
                nc.sync.dma_start(out=mask_t[:],
                                  in_=mask_d[:, int(offC[t]):int(offC[t + 1])])
                rdeg_t = phb.tile([P, 1], f32, name="rdeg_t", tag="rdeg_t")
                nc.sync.dma_start(out=rdeg_t[:],
                                  in_=rdeg_d[t * P:(t + 1) * P, None])

                # slot rows: h = xs @ Wb per column, 3 columns per PSUM bank
                g = phb.tile([P, C, FU], bf16, name="g", tag="g")
                nc.vector.tensor_copy(out=g[:, 0, :], in_=hx_own[:, t, :])
                for k0 in range(1, C, KC):
                    nk = min(KC, C - k0)
                    hp = h_ps.tile([P, KC * FU], f32, name="hp", tag="hp")
                    for i in range(nk):
                        nc.tensor.matmul(
                            out=hp[:, i * FU:(i + 1) * FU],
                            lhsT=xs_t[:, (k0 - 1 + i) * P:(k0 + i) * P],
                            rhs=Wb_s[:], start=True, stop=True)
                    nc.scalar.copy(
                        out=g[:, k0:k0 + nk, :],
                        in_=hp[:, :nk * FU].rearrange("p (k f) -> p k f",
                                                      k=nk))

                # a_edge: quad matmuls [4ED,P] @ [4ED,4H]
                ae = phb.tile([P, C, H], f32, name="ae", tag="ae")
                QG = 16
                for qg in range(math.ceil(C4 / QG)):
                    nq = min(QG, C4 - qg * QG)
                    ae_p = phb_ps.tile([P, QG * 4 * H], f32, name="ae_p",
                                       tag="ae_p")
                    for qi in range(nq):
                        q = qg * QG + qi
                        nc.tensor.matmul(
                            out=ae_p[:, qi * 4 * H:(qi + 1) * 4 * H],
                            lhsT=ea4_t[:, q * P:(q + 1) * P],
                            rhs=v4_s[:],
                            start=True, stop=True)
                    nc.scalar.copy(
                        out=ae[:, qg * QG * 4:qg * QG * 4 + nq * 4, :],
                        in_=ae_p[:, :nq * 4 * H])
                # self-loop a_edge = (sum of a_edge over all slots) / deg
                aeL = phb.tile([P, H], f32, name="aeL", tag="aeL")
                nc.vector.tensor_reduce(
                    out=aeL[:], in_=ae.transpose([0, 2, 1]),
                    axis=mybir.AxisListType.X, op=OP.add)
                nc.vector.tensor_scalar_mul(ae[:, 0, :], aeL[:], rdeg_t[:])

                # logits -> exp(leaky) ; no segment max (clamped at -88)
                asl = phb.tile([P, C, H], f32, name="asl", tag="asl")
                nc.scalar.copy(out=asl[:], in_=g[:, :, F:F + H])
                al = phb.tile([P, H, C], f32, name="al", tag="al")
                alv = al.transpose([0, 2, 1])
                nc.vector.tensor_tensor(
                    out=alv, in0=asl[:], in1=ae[:, :, :], op=OP.add)
                adst = hx_own[:, t, F + H:F + 2 * H]
                nc.vector.tensor_tensor(
                    out=alv, in0=alv,
                    in1=adst.unsqueeze(1).broadcast_to((P, C, H)),
                    op=OP.add)
                nc.vector.tensor_tensor(
                    out=alv, in0=alv,
                    in1=mask_t.unsqueeze(2).broadcast_to((P, C, H)),
                    op=OP.add)
                nc.vector.scalar_tensor_tensor(
                    out=al[:], in0=al[:], scalar=NEG, in1=al[:],
                    op0=OP.mult, op1=OP.max)
                nc.vector.tensor_scalar_max(al[:], al[:], -88.0)
                al16 = phb.tile([P, H, C], bf16, name="al16", tag="al16")
                nc.scalar.activation(out=al16[:], in_=al[:], func=AF.Exp)

                den = phb.tile([P, H], f32, name="den", tag="den")
                nc.vector.tensor_reduce(
                    out=den[:], in_=al16[:],
                    axis=mybir.AxisListType.X, op=OP.add)
                # alternate the big per-slot multiply between DVE and the
                # otherwise-idle GPSIMD engine to overlap tiles (GPSIMD
                # cannot do free-dim reduces, so those stay on DVE)
                eng = nc.gpsimd if t % 2 == 1 else nc.vector
                gh = g[:, :, 0:F].rearrange("p k (h d) -> p k h d", h=H)
                eng.tensor_tensor(
                    out=gh, in0=gh,
                    in1=al16.transpose([0, 2, 1]).unsqueeze(3)
                        .broadcast_to((P, C, H, F // H)),
                    op=OP.mult)
                msg = phb.tile([P, F], f32, name="msg", tag="msg")
                nc.vector.tensor_reduce(
                    out=msg[:],
                    in_=g[:, :, 0:F].transpose([0, 2, 1]),
                    axis=mybir.AxisListType.X, op=OP.add)

                # out_pre = msg / den  (per-node alpha normalization)
                rec = phb.tile([P, H], f32, name="rec", tag="rec")
                nc.vector.tensor_scalar_add(rec[:], den[:], 1e-16)
                nc.vector.reciprocal(rec[:], rec[:])
                op_t = out_all[:, t, :]
                eng.tensor_tensor(
                    out=op_t.rearrange("p (h d) -> p h d", h=H),
                    in0=msg.rearrange("p (h d) -> p h d", h=H),
                    in1=rec.unsqueeze(2).broadcast_to((P, H, F // H)),
                    op=OP.mult)

                # stats: per-channel sum & sumsq via ones-matmul
                sq = phb.tile([P, F], f32, name="sq", tag="sq")
                eng.tensor_tensor(out=sq[:], in0=op_t, in1=op_t, op=OP.mult)
                st_p = stats_ps.tile([P, 2], f32, name="st_p", tag="st_p")
                nc.tensor.matmul(out=st_p[:, 0:1], lhsT=op_t, rhs=ones[:],
                                 start=True, stop=True)
                nc.tensor.matmul(out=st_p[:, 1:2], lhsT=sq[:], rhs=ones[:],
                                 start=True, stop=True)
                nc.vector.tensor_add(acc[:], acc[:], st_p[:])

            # ---------------- Phase C: stats allreduce + normalize + ELU
            st_in = dram.tile([P, 2], f32)
            st_out = dram.tile([P, 2], f32, addr_space="Shared")
            nc.sync.dma_start(out=st_in[:], in_=acc[:])
            nc.gpsimd.collective_compute(
                "AllReduce", OP.add,
                replica_groups=[list(range(NC))],
                ins=[st_in[:].opt()], outs=[st_out[:].opt()])
            sg = keep.tile([P, 2], f32)
            nc.sync.dma_start(out=sg[:], in_=st_out[:])
            mean = keep.tile([P, 1], f32)
            nc.vector.tensor_scalar_mul(mean[:], sg[:, 0:1], 1.0 / N)
            ex2 = keep.tile([P, 1], f32)
            nc.vector.tensor_scalar_mul(ex2[:], sg[:, 1:2], 1.0 / N)
            var = keep.tile([P, 1], f32)
            nc.vector.tensor_mul(var[:], mean[:], mean[:])
            nc.vector.tensor_sub(var[:], ex2[:], var[:])
            rstd = keep.tile([P, 1], f32)
            eps_t = keep.tile([P, 1], f32)
            nc.vector.memset(eps_t[:], EPS_IN)
            nc.scalar.activation(out=rstd[:], in_=var[:], func=AF.Sqrt,
                                 bias=eps_t[:])
            nc.vector.reciprocal(rstd[:], rstd[:])
            gam_s = keep.tile([P, 1], f32)
            nc.sync.dma_start(out=gam_s[:], in_=gam_d[:, None])
            bet_s = keep.tile([P, 1], f32)
            nc.sync.dma_start(out=bet_s[:], in_=bet_d[:, None])
            scl = keep.tile([P, 1], f32)
            nc.vector.tensor_mul(scl[:], rstd[:], gam_s[:])
            bia = keep.tile([P, 1], f32)
            nc.vector.tensor_mul(bia[:], mean[:], scl[:])
            nc.vector.tensor_sub(bia[:], bet_s[:], bia[:])
            sb_dram = dram.tile([2, P], f32)
            nc.sync.dma_start(out=sb_dram[0, :], in_=scl[:, 0])
            nc.sync.dma_start(out=sb_dram[1, :], in_=bia[:, 0])
            sclB = keep.tile([P, F], f32)
            nc.sync.dma_start(out=sclB[:],
                              in_=sb_dram[0:1, :].broadcast_to((P, P)))
            biaB = keep.tile([P, F], f32)
            nc.sync.dma_start(out=biaB[:],
                              in_=sb_dram[1:2, :].broadcast_to((P, P)))

            with tc.tile_pool(name="ph_c", bufs=3) as phc:
                for t in range(n_tiles):
                    xo_t = phc.tile([P, F], f32, name="xo_t")
                    nc.sync.dma_start(out=xo_t[:],
                                      in_=xo_d[t * P:(t + 1) * P, :])
                    z = phc.tile([P, F], f32, name="z")
                    nc.vector.tensor_mul(z[:], out_all[:, t, :], sclB[:])
                    nc.vector.tensor_add(z[:], z[:], biaB[:])
                    nc.vector.tensor_add(z[:], z[:], xo_t[:])
                    zm = phc.tile([P, F], f32, name="zm")
                    nc.vector.tensor_scalar_min(zm[:], z[:], 0.0)
                    nc.scalar.activation(out=zm[:], in_=zm[:], func=AF.Exp)
                    nc.vector.tensor_scalar_max(z[:], z[:], 0.0)
                    nc.vector.tensor_add(z[:], z[:], zm[:])
                    nc.vector.tensor_scalar_add(z[:], z[:], -1.0)
                    nc.sync.dma_start(out=out_d[t * P:(t + 1) * P, :], in_=z[:])
    if finalize:
        nc.finalize()
    return nc


# ---------------------------------------------------------------- driver
def _to_bf16(a):
    import ml_dtypes
    return a.astype(ml_dtypes.bfloat16)


def _run_gat(x, edge_index, edge_attr, W, att_src, att_dst, W_e, att_edge,
             gamma, beta, cfg, trace=False, return_results=False):
    from concourse.bass_utils import run_bass_kernel_spmd

    N, F, H, Dh, NC = cfg["N"], cfg["F"], cfg["H"], cfg["Dh"], cfg["NC"]
    Np = N // NC
    Wb, v4 = _fold_weights(
        np.asarray(W, np.float32), np.asarray(att_src, np.float32),
        np.asarray(att_dst, np.float32), np.asarray(W_e, np.float32),
        np.asarray(att_edge, np.float32), H, Dh)
    x_np = np.asarray(x, np.float32)
    cores, meta = _preprocess(x_np, edge_index, edge_attr, cfg)
    nc = _build(cfg, meta)

    gam = np.asarray(gamma, np.float32)
    bet = np.asarray(beta, np.float32)
    n_tiles = len(meta["Cs"])
    Wb16 = _to_bf16(Wb)
    in_maps = []
    for c in range(NC):
        st = cores[c]["in"]
        order = cores[c]["node_order"]
        gl = np.where(order >= 0, c * Np + order, 0)
        xo = x_np[gl]
        xo[order < 0] = 0.0
        xsTo = _to_bf16(np.ascontiguousarray(xo.T))
        in_maps.append(dict(
            xsT=_to_bf16(st["xsT"]), xsTo=xsTo,
            xo=np.ascontiguousarray(xo), Wb=Wb16, v4=v4,
            eaT4=st["eaT4"], mask=st["mask"], rdeg=st["rdeg"],
            gamma=gam, beta=bet))
    res = run_bass_kernel_spmd(nc, in_maps, core_ids=list(range(NC)),
                               trace=trace)
    out = np.empty((N, F), dtype=np.float32)
    for c in range(NC):
        oc = res.results[c]["out"]
        order = cores[c]["node_order"]
        real = order >= 0
        out[c * Np + order[real]] = oc[np.nonzero(real)[0]]
    if return_results:
        return out, res
    return out


def kernel(x, edge_index, edge_attr, W, att_src, att_dst, W_e, att_edge,
           gamma, beta):
    return _run_gat(x, edge_index, edge_attr, W, att_src, att_dst, W_e,
                    att_edge, gamma, beta, _cfg_full())
